# revision 14
# baseline (speedup 1.0000x reference)
"""Trainium2 Bass kernel for HCEN forward: out = ((x.mean(axis=1)) @ W_enc.T + b_enc) @ W_out.T + b_out.

Sharding: data-parallel over batch. B=16 across 8 cores -> 2 batches/core.
Weights replicated per core. No collectives.

Key ideas (network is fully linear in x, tolerance 2e-2):
  * x ships as bf16 (host cast) -> halves the dominant HBM stream
    (32 -> 16 MiB/core). Mean-of-4096 washes the rounding error out.
  * The two layers collapse: out = m @ C + bias_c with
    C = W_enc.T @ W_out.T and bias_c = b_enc @ W_out.T + b_out.
    C (1024x1024) is built ON DEVICE by the otherwise-idle PE engine
    while x streams (also keeps the PE HAM clock-gate warm), so the
    post-stream tail is just: last fold + ones-matmul + one 16-matmul
    pass (mT.T @ C) + a K=1 ones-matmul for b_out + out DMA.
  * bias_c accumulates directly into the out PSUM group during the
    stream: b_enc ships duplicated into 2 stationary columns so both
    batch rows receive it; b_out is added via a K=1 ones-matmul at the
    end of the group. No extra PSUM banks, no DVE work.
  * Mean pipeline per 2 MiB x tile: first fold stage (free-dim 4096
    bf16 add) on GpSimd, remaining stages (2048/1024 bf16 + mixed add
    into f32 acc) on DVE -- splitting keeps both engines ~50% busy so
    folds track the stream instead of lagging it.
  * Per batch: acc -> bf16 cast, then 8 stationary-acc ones-matmuls
    produce mT [128, 8] directly in PSUM (single bf16 pass).
  * Weight DMAs go after the FIRST x tile; batch 1's last tile is
    split into 2x1 MiB so the end-of-stream fold chain is short.
  * PE FIFO order: C(n0) | C(n1) | bias | mt-b0 | warm-keeper matmuls
    gated on late x tiles (so HAM doesn't re-throttle the PE before
    the tail) | mt-b1 | final.
"""

import os
import sys
from contextlib import ExitStack

import ml_dtypes
import numpy as np

for _p in ("/opt/trn_rl_repo", "/root/.axon_site/_ro/trn_rl_repo"):
    if os.path.isdir(_p) and _p not in sys.path:
        sys.path.insert(0, _p)

import concourse.bass as bass  # noqa: E402
import concourse.tile as tile  # noqa: E402
from concourse import bacc, mybir  # noqa: E402
from concourse.bass_utils import run_bass_kernel_spmd  # noqa: E402

B, S, D, H, O = 16, 4096, 1024, 1024, 1024
NCORES = 8
BPC = B // NCORES  # batches per core
P = 128
DC = D // P
HC = H // P
NF = 512  # matmul moving free dim (PSUM bank limit)
F32 = mybir.dt.float32
F16 = mybir.dt.float16

# per-batch tile plan: rows per tile in units of 128-row groups
# (2 MiB tiles except the last two 1 MiB ones for batch 1)
TILES_B0 = [8, 8, 8, 8]
TILES_B1 = [8, 8, 8, 4, 4]

_CACHE = {}


def build_nc():
    if "nc" in _CACHE:
        return _CACHE["nc"]
    nc = bacc.Bacc(
        "TRN2",
        target_bir_lowering=False,
        debug=False,
        enable_asserts=False,
        num_devices=NCORES,
    )
    x_ext = nc.dram_tensor("x", [BPC, S, D], F16, kind="ExternalInput").ap()
    wenc_ext = nc.dram_tensor("wenc", [H, D], F16, kind="ExternalInput").ap()
    woutT_ext = nc.dram_tensor("woutT", [H, O], F16, kind="ExternalInput").ap()
    bencT2_ext = nc.dram_tensor("bencT2", [P, HC, BPC], F16, kind="ExternalInput").ap()
    bout_ext = nc.dram_tensor("bout", [1, O], F16, kind="ExternalInput").ap()
    out_ext = nc.dram_tensor("out", [BPC, O], F32, kind="ExternalOutput").ap()

    with ExitStack() as ctx:
        tc = ctx.enter_context(tile.TileContext(nc))
        consts = ctx.enter_context(tc.tile_pool(name="consts", bufs=1))
        wpool = ctx.enter_context(tc.tile_pool(name="wpool", bufs=1))
        xpool = ctx.enter_context(tc.tile_pool(name="xpool", bufs=6))
        apool = ctx.enter_context(tc.tile_pool(name="apool", bufs=1))
        spool = ctx.enter_context(tc.tile_pool(name="spool", bufs=1))
        cpp = ctx.enter_context(tc.tile_pool(name="cpp", bufs=4, space="PSUM"))
        mtp = ctx.enter_context(tc.tile_pool(name="mtp", bufs=2, space="PSUM"))
        opp = ctx.enter_context(tc.tile_pool(name="opp", bufs=1, space="PSUM"))

        ones_bf = consts.tile([P, 1], F16)
        nc.gpsimd.memset(ones_bf[:], 1.0)
        ones1_bf = consts.tile([1, BPC], F16)
        nc.gpsimd.memset(ones1_bf[:], 1.0)

        # ---- DMA program order: 1 x tile, all weights, rest of x ----
        plans = [TILES_B0, TILES_B1]
        xts = [[None] * len(p) for p in plans]

        def issue_x(b, t):
            qt = plans[b][t]
            row0 = 128 * sum(plans[b][:t])
            xt = xpool.tile([P, qt, D], F16, name="xt", tag="xt")
            nc.sync.dma_start(
                xt[:],
                x_ext[b, row0 : row0 + P * qt, :].rearrange("(p q) d -> p q d", p=P),
            )
            xts[b][t] = xt

        issue_x(0, 0)

        wenc_sb = wpool.tile([P, HC, D], F16)
        nc.sync.dma_start(wenc_sb[:], wenc_ext.rearrange("(c p) d -> p c d", p=P))
        wout_sb = wpool.tile([P, HC, O], F16)
        nc.sync.dma_start(wout_sb[:], woutT_ext.rearrange("(c p) d -> p c d", p=P))
        bencT2_sb = consts.tile([P, HC, BPC], F16)
        nc.sync.dma_start(bencT2_sb[:], bencT2_ext[:])
        bout_sb = consts.tile([1, O], F16)
        nc.sync.dma_start(bout_sb[:], bout_ext[:])

        for t in range(1, len(plans[0])):
            issue_x(0, t)
        for t in range(len(plans[1])):
            issue_x(1, t)

        # ---- fold pipeline: DVE tree folds, two independent acc chains
        #      per batch (even/odd tiles) so the end-of-stream serial chain
        #      is short; the mt ones-matmul pass sums both chains ----
        accs = [
            [apool.tile([P, D], F16, name=f"acc{b}_{j}") for j in range(2)]
            for b in range(BPC)
        ]
        mt_sb = spool.tile([P, DC, BPC], F16)

        def issue_folds(b):
            for t, qt in enumerate(plans[b]):
                xt = xts[b][t]
                h = qt // 2
                nc.vector.tensor_add(xt[:, 0:h, :], xt[:, 0:h, :], xt[:, h:qt, :])
                while h > 1:
                    nc.vector.tensor_add(
                        xt[:, 0 : h // 2, :], xt[:, 0 : h // 2, :], xt[:, h // 2 : h, :]
                    )
                    h //= 2
                acc = accs[b][t % 2]
                if t < 2:
                    nc.vector.tensor_copy(acc[:], xt[:, 0, :])
                else:
                    nc.vector.tensor_add(acc[:], acc[:], xt[:, 0, :])

        def issue_mt(b):
            # PE: mT[d, b] = column sums of both acc chains (ones-matmuls
            # accumulating in PSUM)
            mt_ps = mtp.tile([P, DC], F32, name=f"mtps{b}", tag="mtps")
            for c in range(DC):
                for j in range(2):
                    nc.tensor.matmul(
                        mt_ps[:, c : c + 1],
                        accs[b][j][:, c * P : (c + 1) * P],
                        ones_bf[:],
                        start=(j == 0),
                        stop=(j == 1),
                        skip_group_check=True,
                    )
            nc.scalar.mul(mt_sb[:, :, b], mt_ps[:], 1.0 / S)

        c_sb = wpool.tile([P, DC, O], F16)

        def issue_c_block(ds):
            # PE: C[d-chunks, :] = (W_enc.T @ W_out.T) rows; one LDWEIGHTS
            # per (d, h) feeds both n-half matmuls (separate 1-bank PSUM
            # tiles per half -- PSUM matmul targets must not span banks)
            for d in ds:
                c_pss = [
                    cpp.tile([P, NF], F32, name=f"cps{d}_{n}", tag="cps")
                    for n in range(O // NF)
                ]
                for h in range(HC):
                    for n in range(O // NF):
                        nc.tensor.matmul(
                            c_pss[n][:],
                            wenc_sb[:, h, d * P : (d + 1) * P],
                            wout_sb[:, h, n * NF : (n + 1) * NF],
                            start=(h == 0),
                            stop=(h == HC - 1),
                            skip_group_check=True,
                        )
                for n in range(O // NF):
                    nc.scalar.copy(
                        c_sb[:, d, n * NF : (n + 1) * NF], c_pss[n][:]
                    )

        issue_folds(0)
        issue_folds(1)

        issue_c_block(range(4))
        issue_c_block(range(4, DC))

        # PE: bias rows b_enc @ W_out.T accumulate directly into out_ps
        # (b_enc duplicated into BPC stationary columns -> lands in both rows)
        out_ps = opp.tile([BPC, O], F32, name="outps")
        for n in range(O // NF):
            for h in range(HC):
                nc.tensor.matmul(
                    out_ps[:, n * NF : (n + 1) * NF],
                    bencT2_sb[:, h, :],
                    wout_sb[:, h, n * NF : (n + 1) * NF],
                    start=(h == 0),
                    stop=False,
                    skip_group_check=True,
                )

        issue_mt(0)

        # warm-keeper matmuls gated on late b1 tiles: keep the PE HAM
        # clock-gate from re-throttling during the pre-tail DMA stretch
        for t in range(2, len(plans[1])):
            warm_ps = cpp.tile([1, NF], F32, name=f"warm{t}", tag="cps")
            nc.tensor.matmul(
                warm_ps[:],
                ones_bf[:],
                xts[1][t][:, 0, 0:NF],
                skip_group_check=True,
            )
        # ungated fillers: burn the PE FIFO gap while the DVE fold queue
        # drains so the final pass runs at the warm clock
        for k in range(12):
            warm_ps = cpp.tile([1, NF], F32, name=f"fill{k}", tag="cps")
            nc.tensor.matmul(
                warm_ps[:],
                ones_bf[:],
                wout_sb[:, k % HC, 0:NF],
                skip_group_check=True,
            )

        issue_mt(1)

        # ---- tail: out_ps += mT.T @ C, then += ones1.T @ b_out ----
        out_sb = spool.tile([BPC, O], F32)
        for n in range(O // NF):
            sl = slice(n * NF, (n + 1) * NF)
            for c in range(DC):
                nc.tensor.matmul(
                    out_ps[:, sl],
                    mt_sb[:, c, :],
                    c_sb[:, c, sl],
                    start=False,
                    stop=False,
                    skip_group_check=True,
                )
            nc.tensor.matmul(
                out_ps[:, sl],
                ones1_bf[:],
                bout_sb[:, sl],
                start=False,
                stop=True,
                skip_group_check=True,
            )
            nc.scalar.copy(out_sb[:, sl], out_ps[:, sl])
            nc.sync.dma_start(out_ext[:, sl], out_sb[:, sl])

    nc.compile()
    _CACHE["nc"] = nc
    return nc


def make_in_maps(x, W_enc, b_enc, W_out, b_out):
    xb = np.asarray(x, dtype=np.float32).astype(np.float16)
    wenc = np.ascontiguousarray(np.asarray(W_enc, dtype=np.float32).astype(np.float16))
    woutT = np.ascontiguousarray(
        np.asarray(W_out, dtype=np.float32).T.astype(np.float16)
    )
    bencT = np.asarray(b_enc, dtype=np.float32).reshape(HC, P).T
    bencT2 = np.ascontiguousarray(
        np.repeat(bencT[:, :, None], BPC, axis=2).astype(np.float16)
    )
    bout = np.ascontiguousarray(
        np.asarray(b_out, dtype=np.float32).reshape(1, O).astype(np.float16)
    )
    return [
        {
            "x": np.ascontiguousarray(xb[i * BPC : (i + 1) * BPC]),
            "wenc": wenc,
            "woutT": woutT,
            "bencT2": bencT2,
            "bout": bout,
        }
        for i in range(NCORES)
    ]


def gather_out(results):
    return np.ascontiguousarray(
        np.concatenate([results[i]["out"] for i in range(NCORES)], axis=0)
    )


def kernel(x, W_enc, b_enc, W_out, b_out):
    nc = build_nc()
    in_maps = make_in_maps(x, W_enc, b_enc, W_out, b_out)
    res = run_bass_kernel_spmd(nc, in_maps, list(range(NCORES)))
    return gather_out(res.results)


# revision 15
# speedup vs baseline: 1.1019x; 1.1019x over previous
"""Trainium2 Bass kernel for HCEN forward: out = ((x.mean(axis=1)) @ W_enc.T + b_enc) @ W_out.T + b_out.

Sharding: data-parallel over batch. B=16 across 8 cores -> 2 batches/core.
Weights replicated per core. No collectives.

Key ideas (network is fully linear in x, tolerance 2e-2):
  * x ships as bf16 (host cast) -> halves the dominant HBM stream
    (32 -> 16 MiB/core). Mean-of-4096 washes the rounding error out.
  * The two layers collapse: out = m @ C + bias_c with
    C = W_enc.T @ W_out.T and bias_c = b_enc @ W_out.T + b_out.
    C (1024x1024) is built ON DEVICE by the otherwise-idle PE engine
    while x streams (also keeps the PE HAM clock-gate warm), so the
    post-stream tail is just: last fold + ones-matmul + one 16-matmul
    pass (mT.T @ C) + a K=1 ones-matmul for b_out + out DMA.
  * bias_c accumulates directly into the out PSUM group during the
    stream: b_enc ships duplicated into 2 stationary columns so both
    batch rows receive it; b_out is added via a K=1 ones-matmul at the
    end of the group. No extra PSUM banks, no DVE work.
  * Mean pipeline per 2 MiB x tile: first fold stage (free-dim 4096
    bf16 add) on GpSimd, remaining stages (2048/1024 bf16 + mixed add
    into f32 acc) on DVE -- splitting keeps both engines ~50% busy so
    folds track the stream instead of lagging it.
  * Per batch: acc -> bf16 cast, then 8 stationary-acc ones-matmuls
    produce mT [128, 8] directly in PSUM (single bf16 pass).
  * Weight DMAs go after the FIRST x tile; batch 1's last tile is
    split into 2x1 MiB so the end-of-stream fold chain is short.
  * PE FIFO order: C(n0) | C(n1) | bias | mt-b0 | warm-keeper matmuls
    gated on late x tiles (so HAM doesn't re-throttle the PE before
    the tail) | mt-b1 | final.
"""

import os
import sys
from contextlib import ExitStack

import ml_dtypes
import numpy as np

for _p in ("/opt/trn_rl_repo", "/root/.axon_site/_ro/trn_rl_repo"):
    if os.path.isdir(_p) and _p not in sys.path:
        sys.path.insert(0, _p)

import concourse.bass as bass  # noqa: E402
import concourse.tile as tile  # noqa: E402
from concourse import bacc, mybir  # noqa: E402
from concourse.bass_utils import run_bass_kernel_spmd  # noqa: E402

B, S, D, H, O = 16, 4096, 1024, 1024, 1024
NCORES = 8
BPC = B // NCORES  # batches per core
P = 128
DC = D // P
HC = H // P
NF = 512  # matmul moving free dim (PSUM bank limit)
F32 = mybir.dt.float32
F16 = mybir.dt.float16

# per-batch tile plan: rows per tile in units of 128-row groups
# (2 MiB tiles except the last two 1 MiB ones for batch 1)
TILES_B0 = [8, 8, 8, 8]
TILES_B1 = [8, 8, 8, 4, 4]

_CACHE = {}


def build_nc():
    if "nc" in _CACHE:
        return _CACHE["nc"]
    nc = bacc.Bacc(
        "TRN2",
        target_bir_lowering=False,
        debug=False,
        enable_asserts=False,
        num_devices=NCORES,
    )
    x_ext = nc.dram_tensor("x", [BPC, S, D], F16, kind="ExternalInput").ap()
    wenc_ext = nc.dram_tensor("wenc", [H, D], F16, kind="ExternalInput").ap()
    woutT_ext = nc.dram_tensor("woutT", [H, O], F16, kind="ExternalInput").ap()
    bencT2_ext = nc.dram_tensor("bencT2", [P, HC, BPC], F16, kind="ExternalInput").ap()
    bout_ext = nc.dram_tensor("bout", [1, O], F16, kind="ExternalInput").ap()
    out_ext = nc.dram_tensor("out", [BPC, O], F32, kind="ExternalOutput").ap()

    with ExitStack() as ctx:
        tc = ctx.enter_context(tile.TileContext(nc))
        consts = ctx.enter_context(tc.tile_pool(name="consts", bufs=1))
        wpool = ctx.enter_context(tc.tile_pool(name="wpool", bufs=1))
        xpool = ctx.enter_context(tc.tile_pool(name="xpool", bufs=6))
        apool = ctx.enter_context(tc.tile_pool(name="apool", bufs=1))
        spool = ctx.enter_context(tc.tile_pool(name="spool", bufs=1))
        cpp = ctx.enter_context(tc.tile_pool(name="cpp", bufs=4, space="PSUM"))
        mtp = ctx.enter_context(tc.tile_pool(name="mtp", bufs=2, space="PSUM"))
        opp = ctx.enter_context(tc.tile_pool(name="opp", bufs=1, space="PSUM"))

        ones_bf = consts.tile([P, 1], F16)
        nc.gpsimd.memset(ones_bf[:], 1.0)
        ones1_bf = consts.tile([1, BPC], F16)
        nc.gpsimd.memset(ones1_bf[:], 1.0)

        # ---- DMA program order: 1 x tile, all weights, rest of x ----
        plans = [TILES_B0, TILES_B1]
        xts = [[None] * len(p) for p in plans]

        def issue_x(b, t):
            qt = plans[b][t]
            row0 = 128 * sum(plans[b][:t])
            xt = xpool.tile([P, qt, D], F16, name="xt", tag="xt")
            nc.sync.dma_start(
                xt[:],
                x_ext[b, row0 : row0 + P * qt, :].rearrange("(p q) d -> p q d", p=P),
            )
            xts[b][t] = xt

        issue_x(0, 0)

        wenc_sb = wpool.tile([P, HC, D], F16)
        nc.sync.dma_start(wenc_sb[:], wenc_ext.rearrange("(c p) d -> p c d", p=P))
        wout_sb = wpool.tile([P, HC, O], F16)
        nc.sync.dma_start(wout_sb[:], woutT_ext.rearrange("(c p) d -> p c d", p=P))
        bencT2_sb = consts.tile([P, HC, BPC], F16)
        nc.sync.dma_start(bencT2_sb[:], bencT2_ext[:])
        bout_sb = consts.tile([1, O], F16)
        nc.sync.dma_start(bout_sb[:], bout_ext[:])

        for t in range(1, len(plans[0])):
            issue_x(0, t)
        for t in range(len(plans[1])):
            issue_x(1, t)

        # ---- fold pipeline: DVE tree folds, two independent acc chains
        #      per batch (even/odd tiles) so the end-of-stream serial chain
        #      is short; the mt ones-matmul pass sums both chains ----
        accs = [
            [apool.tile([P, D], F16, name=f"acc{b}_{j}") for j in range(2)]
            for b in range(BPC)
        ]
        mt_sb = spool.tile([P, DC, BPC], F16)

        def issue_folds(b):
            for t, qt in enumerate(plans[b]):
                xt = xts[b][t]
                h = qt // 2
                nc.vector.tensor_add(xt[:, 0:h, :], xt[:, 0:h, :], xt[:, h:qt, :])
                while h > 1:
                    nc.vector.tensor_add(
                        xt[:, 0 : h // 2, :], xt[:, 0 : h // 2, :], xt[:, h // 2 : h, :]
                    )
                    h //= 2
                acc = accs[b][t % 2]
                if t < 2:
                    nc.vector.tensor_copy(acc[:], xt[:, 0, :])
                else:
                    nc.vector.tensor_add(acc[:], acc[:], xt[:, 0, :])

        def issue_mt(b):
            # PE: mT[d, b] = column sums of both acc chains (ones-matmuls
            # accumulating in PSUM)
            mt_ps = mtp.tile([P, DC], F32, name=f"mtps{b}", tag="mtps")
            for c in range(DC):
                for j in range(2):
                    nc.tensor.matmul(
                        mt_ps[:, c : c + 1],
                        accs[b][j][:, c * P : (c + 1) * P],
                        ones_bf[:],
                        start=(j == 0),
                        stop=(j == 1),
                        skip_group_check=True,
                    )
            nc.scalar.mul(mt_sb[:, :, b], mt_ps[:], 1.0 / S)

        c_sb = wpool.tile([P, DC, O], F16)

        def issue_c_block(ds):
            # PE: C[d-chunks, :] = (W_enc.T @ W_out.T) rows; one LDWEIGHTS
            # per (d, h) feeds both n-half matmuls (separate 1-bank PSUM
            # tiles per half -- PSUM matmul targets must not span banks)
            for d in ds:
                c_pss = [
                    cpp.tile([P, NF], F32, name=f"cps{d}_{n}", tag="cps")
                    for n in range(O // NF)
                ]
                for h in range(HC):
                    for n in range(O // NF):
                        nc.tensor.matmul(
                            c_pss[n][:],
                            wenc_sb[:, h, d * P : (d + 1) * P],
                            wout_sb[:, h, n * NF : (n + 1) * NF],
                            start=(h == 0),
                            stop=(h == HC - 1),
                            skip_group_check=True,
                        )
                for n in range(O // NF):
                    nc.scalar.copy(
                        c_sb[:, d, n * NF : (n + 1) * NF], c_pss[n][:]
                    )

        issue_folds(0)
        issue_folds(1)

        issue_c_block(range(4))
        issue_c_block(range(4, DC))

        # PE: bias rows b_enc @ W_out.T accumulate directly into out_ps
        # (b_enc duplicated into BPC stationary columns -> lands in both rows)
        out_ps = opp.tile([BPC, O], F32, name="outps")
        for n in range(O // NF):
            for h in range(HC):
                nc.tensor.matmul(
                    out_ps[:, n * NF : (n + 1) * NF],
                    bencT2_sb[:, h, :],
                    wout_sb[:, h, n * NF : (n + 1) * NF],
                    start=(h == 0),
                    stop=False,
                    skip_group_check=True,
                )

        issue_mt(0)

        # warm-keeper matmuls gated on late b1 tiles: keep the PE HAM
        # clock-gate from re-throttling during the pre-tail DMA stretch
        for t in range(2, len(plans[1])):
            warm_ps = cpp.tile([1, NF], F32, name=f"warm{t}", tag="cps")
            nc.tensor.matmul(
                warm_ps[:],
                ones_bf[:],
                xts[1][t][:, 0, 0:NF],
                skip_group_check=True,
            )

        issue_mt(1)

        # ---- tail: out_ps += mT.T @ C, then += ones1.T @ b_out ----
        out_sb = spool.tile([BPC, O], F32)
        for n in range(O // NF):
            sl = slice(n * NF, (n + 1) * NF)
            for c in range(DC):
                nc.tensor.matmul(
                    out_ps[:, sl],
                    mt_sb[:, c, :],
                    c_sb[:, c, sl],
                    start=False,
                    stop=False,
                    skip_group_check=True,
                )
            nc.tensor.matmul(
                out_ps[:, sl],
                ones1_bf[:],
                bout_sb[:, sl],
                start=False,
                stop=True,
                skip_group_check=True,
            )
            nc.scalar.copy(out_sb[:, sl], out_ps[:, sl])
            nc.sync.dma_start(out_ext[:, sl], out_sb[:, sl])

    nc.compile()
    _CACHE["nc"] = nc
    return nc


def make_in_maps(x, W_enc, b_enc, W_out, b_out):
    xb = np.asarray(x, dtype=np.float32).astype(np.float16)
    wenc = np.ascontiguousarray(np.asarray(W_enc, dtype=np.float32).astype(np.float16))
    woutT = np.ascontiguousarray(
        np.asarray(W_out, dtype=np.float32).T.astype(np.float16)
    )
    bencT = np.asarray(b_enc, dtype=np.float32).reshape(HC, P).T
    bencT2 = np.ascontiguousarray(
        np.repeat(bencT[:, :, None], BPC, axis=2).astype(np.float16)
    )
    bout = np.ascontiguousarray(
        np.asarray(b_out, dtype=np.float32).reshape(1, O).astype(np.float16)
    )
    return [
        {
            "x": np.ascontiguousarray(xb[i * BPC : (i + 1) * BPC]),
            "wenc": wenc,
            "woutT": woutT,
            "bencT2": bencT2,
            "bout": bout,
        }
        for i in range(NCORES)
    ]


def gather_out(results):
    return np.ascontiguousarray(
        np.concatenate([results[i]["out"] for i in range(NCORES)], axis=0)
    )


def kernel(x, W_enc, b_enc, W_out, b_out):
    nc = build_nc()
    in_maps = make_in_maps(x, W_enc, b_enc, W_out, b_out)
    res = run_bass_kernel_spmd(nc, in_maps, list(range(NCORES)))
    return gather_out(res.results)


# revision 16
# speedup vs baseline: 1.2032x; 1.0919x over previous
"""Trainium2 Bass kernel for HCEN forward: out = ((x.mean(axis=1)) @ W_enc.T + b_enc) @ W_out.T + b_out.

Sharding: data-parallel over batch. B=16 across 8 cores -> 2 batches/core.
Weights replicated per core. No collectives.

Key ideas (network is fully linear in x, tolerance 2e-2):
  * x ships as bf16 (host cast) -> halves the dominant HBM stream
    (32 -> 16 MiB/core). Mean-of-4096 washes the rounding error out.
  * The two layers collapse: out = m @ C + bias_c with
    C = W_enc.T @ W_out.T and bias_c = b_enc @ W_out.T + b_out.
    C (1024x1024) is built ON DEVICE by the otherwise-idle PE engine
    while x streams (also keeps the PE HAM clock-gate warm), so the
    post-stream tail is just: last fold + ones-matmul + one 16-matmul
    pass (mT.T @ C) + a K=1 ones-matmul for b_out + out DMA.
  * bias_c accumulates directly into the out PSUM group during the
    stream: b_enc ships duplicated into 2 stationary columns so both
    batch rows receive it; b_out is added via a K=1 ones-matmul at the
    end of the group. No extra PSUM banks, no DVE work.
  * Everything ships fp16 (not bf16): same bytes, 4x finer mantissa,
    and fp16 acc keeps the cross-tile accumulate inside the DVE's
    2x-packed 16-bit mode.
  * Mean pipeline per 2 MiB x tile: DVE tree folds (4096/2048/1024
    free-dim fp16 adds) into two independent acc chains per batch
    (even/odd tiles) so the end-of-stream serial chain is short; the
    mt ones-matmul pass sums both chains per column (one PSUM
    accumulation group open per bank at a time -- hardware constraint).
  * Weight DMAs go after the FIRST x tile; batch 1's last tile is
    split into 2x1 MiB so the end-of-stream fold chain is short.
  * PE FIFO order: C(n0) | C(n1) | bias | mt-b0 | warm-keeper matmuls
    gated on late x tiles (so HAM doesn't re-throttle the PE before
    the tail) | mt-b1 | final.
"""

import os
import sys
from contextlib import ExitStack

import ml_dtypes
import numpy as np

for _p in ("/opt/trn_rl_repo", "/root/.axon_site/_ro/trn_rl_repo"):
    if os.path.isdir(_p) and _p not in sys.path:
        sys.path.insert(0, _p)

import concourse.bass as bass  # noqa: E402
import concourse.tile as tile  # noqa: E402
from concourse import bacc, mybir  # noqa: E402
from concourse.bass_utils import run_bass_kernel_spmd  # noqa: E402

B, S, D, H, O = 16, 4096, 1024, 1024, 1024
NCORES = 8
BPC = B // NCORES  # batches per core
P = 128
DC = D // P
HC = H // P
NF = 512  # matmul moving free dim (PSUM bank limit)
F32 = mybir.dt.float32
F16 = mybir.dt.float16

# per-batch tile plan: rows per tile in units of 128-row groups
# (2 MiB tiles except the last two 1 MiB ones for batch 1)
TILES_B0 = [8, 8, 8, 8]
TILES_B1 = [8, 8, 8, 4, 4]

_CACHE = {}


def build_nc():
    if "nc" in _CACHE:
        return _CACHE["nc"]
    nc = bacc.Bacc(
        "TRN2",
        target_bir_lowering=False,
        debug=False,
        enable_asserts=False,
        num_devices=NCORES,
    )
    x_ext = nc.dram_tensor("x", [BPC, S, D], F16, kind="ExternalInput").ap()
    wenc_ext = nc.dram_tensor("wenc", [H, D], F16, kind="ExternalInput").ap()
    woutT_ext = nc.dram_tensor("woutT", [H, O], F16, kind="ExternalInput").ap()
    bencT2_ext = nc.dram_tensor("bencT2", [P, HC, BPC], F16, kind="ExternalInput").ap()
    bout_ext = nc.dram_tensor("bout", [1, O], F16, kind="ExternalInput").ap()
    out_ext = nc.dram_tensor("out", [BPC, O], F32, kind="ExternalOutput").ap()

    with ExitStack() as ctx:
        tc = ctx.enter_context(tile.TileContext(nc))
        consts = ctx.enter_context(tc.tile_pool(name="consts", bufs=1))
        wpool = ctx.enter_context(tc.tile_pool(name="wpool", bufs=1))
        xpool = ctx.enter_context(tc.tile_pool(name="xpool", bufs=6))
        apool = ctx.enter_context(tc.tile_pool(name="apool", bufs=1))
        spool = ctx.enter_context(tc.tile_pool(name="spool", bufs=1))
        cpp = ctx.enter_context(tc.tile_pool(name="cpp", bufs=4, space="PSUM"))
        mtp = ctx.enter_context(tc.tile_pool(name="mtp", bufs=2, space="PSUM"))
        opp = ctx.enter_context(tc.tile_pool(name="opp", bufs=1, space="PSUM"))

        ones_bf = consts.tile([P, 1], F16)
        nc.gpsimd.memset(ones_bf[:], 1.0)
        ones1_bf = consts.tile([1, BPC], F16)
        nc.gpsimd.memset(ones1_bf[:], 1.0)

        # ---- DMA program order: 1 x tile, all weights, rest of x ----
        plans = [TILES_B0, TILES_B1]
        xts = [[None] * len(p) for p in plans]

        def issue_x(b, t):
            qt = plans[b][t]
            row0 = 128 * sum(plans[b][:t])
            xt = xpool.tile([P, qt, D], F16, name="xt", tag="xt")
            nc.sync.dma_start(
                xt[:],
                x_ext[b, row0 : row0 + P * qt, :].rearrange("(p q) d -> p q d", p=P),
            )
            xts[b][t] = xt

        issue_x(0, 0)

        wenc_sb = wpool.tile([P, HC, D], F16)
        nc.sync.dma_start(wenc_sb[:], wenc_ext.rearrange("(c p) d -> p c d", p=P))
        wout_sb = wpool.tile([P, HC, O], F16)
        nc.sync.dma_start(wout_sb[:], woutT_ext.rearrange("(c p) d -> p c d", p=P))
        bencT2_sb = consts.tile([P, HC, BPC], F16)
        nc.sync.dma_start(bencT2_sb[:], bencT2_ext[:])
        bout_sb = consts.tile([1, O], F16)
        nc.sync.dma_start(bout_sb[:], bout_ext[:])

        for t in range(1, len(plans[0])):
            issue_x(0, t)
        for t in range(len(plans[1])):
            issue_x(1, t)

        # ---- fold pipeline: DVE tree folds, two independent acc chains
        #      per batch (even/odd tiles) so the end-of-stream serial chain
        #      is short; the mt ones-matmul pass sums both chains ----
        accs = [
            [apool.tile([P, D], F16, name=f"acc{b}_{j}") for j in range(2)]
            for b in range(BPC)
        ]
        mt_sb = spool.tile([P, DC, BPC], F16)

        def issue_folds(b):
            for t, qt in enumerate(plans[b]):
                xt = xts[b][t]
                h = qt // 2
                nc.vector.tensor_add(xt[:, 0:h, :], xt[:, 0:h, :], xt[:, h:qt, :])
                while h > 1:
                    nc.vector.tensor_add(
                        xt[:, 0 : h // 2, :], xt[:, 0 : h // 2, :], xt[:, h // 2 : h, :]
                    )
                    h //= 2
                acc = accs[b][t % 2]
                if t < 2:
                    nc.vector.tensor_copy(acc[:], xt[:, 0, :])
                else:
                    nc.vector.tensor_add(acc[:], acc[:], xt[:, 0, :])

        def issue_mt(b):
            # PE: mT[d, b] = column sums of both acc chains (ones-matmuls
            # accumulating in PSUM)
            mt_ps = mtp.tile([P, DC], F32, name=f"mtps{b}", tag="mtps")
            for c in range(DC):
                for j in range(2):
                    nc.tensor.matmul(
                        mt_ps[:, c : c + 1],
                        accs[b][j][:, c * P : (c + 1) * P],
                        ones_bf[:],
                        start=(j == 0),
                        stop=(j == 1),
                        skip_group_check=True,
                    )
            nc.scalar.mul(mt_sb[:, :, b], mt_ps[:], 1.0 / S)

        c_sb = wpool.tile([P, DC, O], F16)

        def issue_c_block(ds):
            # PE: C[d-chunks, :] = (W_enc.T @ W_out.T) rows; one LDWEIGHTS
            # per (d, h) feeds both n-half matmuls (separate 1-bank PSUM
            # tiles per half -- PSUM matmul targets must not span banks)
            for d in ds:
                c_pss = [
                    cpp.tile([P, NF], F32, name=f"cps{d}_{n}", tag="cps")
                    for n in range(O // NF)
                ]
                for h in range(HC):
                    for n in range(O // NF):
                        nc.tensor.matmul(
                            c_pss[n][:],
                            wenc_sb[:, h, d * P : (d + 1) * P],
                            wout_sb[:, h, n * NF : (n + 1) * NF],
                            start=(h == 0),
                            stop=(h == HC - 1),
                            skip_group_check=True,
                        )
                for n in range(O // NF):
                    nc.scalar.copy(
                        c_sb[:, d, n * NF : (n + 1) * NF], c_pss[n][:]
                    )

        issue_folds(0)
        issue_folds(1)

        issue_c_block(range(4))
        issue_c_block(range(4, DC))

        # PE: bias rows b_enc @ W_out.T accumulate directly into out_ps
        # (b_enc duplicated into BPC stationary columns -> lands in both rows)
        out_ps = opp.tile([BPC, O], F32, name="outps")
        for n in range(O // NF):
            for h in range(HC):
                nc.tensor.matmul(
                    out_ps[:, n * NF : (n + 1) * NF],
                    bencT2_sb[:, h, :],
                    wout_sb[:, h, n * NF : (n + 1) * NF],
                    start=(h == 0),
                    stop=False,
                    skip_group_check=True,
                )

        issue_mt(0)

        # warm-keeper matmuls gated on late b1 tiles: keep the PE HAM
        # clock-gate from re-throttling during the pre-tail DMA stretch
        for t in range(2, len(plans[1])):
            warm_ps = cpp.tile([1, NF], F32, name=f"warm{t}", tag="cps")
            nc.tensor.matmul(
                warm_ps[:],
                ones_bf[:],
                xts[1][t][:, 0, 0:NF],
                skip_group_check=True,
            )

        issue_mt(1)

        # ---- tail: out_ps += mT.T @ C, then += ones1.T @ b_out ----
        out_sb = spool.tile([BPC, O], F32)
        for n in range(O // NF):
            sl = slice(n * NF, (n + 1) * NF)
            for c in range(DC):
                nc.tensor.matmul(
                    out_ps[:, sl],
                    mt_sb[:, c, :],
                    c_sb[:, c, sl],
                    start=False,
                    stop=False,
                    skip_group_check=True,
                )
            nc.tensor.matmul(
                out_ps[:, sl],
                ones1_bf[:],
                bout_sb[:, sl],
                start=False,
                stop=True,
                skip_group_check=True,
            )
            nc.scalar.copy(out_sb[:, sl], out_ps[:, sl])
        nc.sync.dma_start(out_ext[:], out_sb[:])

    nc.compile()
    _CACHE["nc"] = nc
    return nc


def make_in_maps(x, W_enc, b_enc, W_out, b_out):
    xb = np.asarray(x, dtype=np.float32).astype(np.float16)
    wenc = np.ascontiguousarray(np.asarray(W_enc, dtype=np.float32).astype(np.float16))
    woutT = np.ascontiguousarray(
        np.asarray(W_out, dtype=np.float32).T.astype(np.float16)
    )
    bencT = np.asarray(b_enc, dtype=np.float32).reshape(HC, P).T
    bencT2 = np.ascontiguousarray(
        np.repeat(bencT[:, :, None], BPC, axis=2).astype(np.float16)
    )
    bout = np.ascontiguousarray(
        np.asarray(b_out, dtype=np.float32).reshape(1, O).astype(np.float16)
    )
    return [
        {
            "x": np.ascontiguousarray(xb[i * BPC : (i + 1) * BPC]),
            "wenc": wenc,
            "woutT": woutT,
            "bencT2": bencT2,
            "bout": bout,
        }
        for i in range(NCORES)
    ]


def gather_out(results):
    return np.ascontiguousarray(
        np.concatenate([results[i]["out"] for i in range(NCORES)], axis=0)
    )


def kernel(x, W_enc, b_enc, W_out, b_out):
    nc = build_nc()
    in_maps = make_in_maps(x, W_enc, b_enc, W_out, b_out)
    res = run_bass_kernel_spmd(nc, in_maps, list(range(NCORES)))
    return gather_out(res.results)
